# revision 25
# baseline (speedup 1.0000x reference)
"""DCRNN (nn_DCRNNModel_35837207118645) Trainium2 Bass kernel, v2.

Data-parallel over batch: B=16 -> 2 per core x 8 cores; 24 recurrent steps
fully unrolled, everything resident in SBUF.

Key structure (vs v1):
  - Split-by-half diffusion: W^k is applied to h0, h1, r*h0, r1*h1 halves
    separately (stationary = node-major halves of BOTH batches packed to
    128 cols).  D(h0_t) computed for layer-1 of step t is reused by
    layer-0 of step t+1, removing one of the four diffusion rounds.
  - r+z gate weights packed side by side -> one matmul stream serves both
    gates; n-gate matmuls for the two batch items write disjoint
    partition halves of one PSUM tile (one tanh for both).
  - Encoder x-hops are computed in 2-step groups (4 stationary columns at
    partitions {0,32,64,96}); decoder x-hops are derived algebraically:
    W^k y = (W^k h1) @ Wp + bp, reusing the already-diffused h1 halves.
"""

from contextlib import ExitStack

import numpy as np

import concourse.bass as bass
import concourse.bacc as bacc
import concourse.tile as tile
import concourse.mybir as mybir
from concourse import masks
from concourse.bass_utils import run_bass_kernel_spmd

FP = mybir.dt.float32
R32 = mybir.dt.float32r


def rr(ap):
    return ap.bitcast(R32)


def rw(ap):
    # round-on-write marker for producers feeding f32r matmuls
    return ap.bitcast(R32)


N = 512
HID = 64
NC = 8          # cores
BPC = 2         # batch per core
T_ENC = 12
T_DEC = 12
NJC = N // 128  # 4 node chunks


def build_module(t_enc=T_ENC, t_dec=T_DEC):
    nc = bacc.Bacc("TRN2", target_bir_lowering=False, debug=False)

    x_d = nc.dram_tensor("x", (BPC, T_ENC, N), FP, kind="ExternalInput").ap()
    A_d = nc.dram_tensor("A", (N, N), FP, kind="ExternalInput").ap()
    wl0, wl1 = {}, {}
    for g in "rzn":
        wl0[g] = nc.dram_tensor(f"W{g}0", (325, HID), FP, kind="ExternalInput").ap()
        wl0[g + "b"] = nc.dram_tensor(f"b{g}0", (HID,), FP, kind="ExternalInput").ap()
        wl1[g] = nc.dram_tensor(f"W{g}1", (640, HID), FP, kind="ExternalInput").ap()
        wl1[g + "b"] = nc.dram_tensor(f"b{g}1", (HID,), FP, kind="ExternalInput").ap()
    Wp_d = nc.dram_tensor("Wp", (HID, 1), FP, kind="ExternalInput").ap()
    bp_d = nc.dram_tensor("bp", (1,), FP, kind="ExternalInput").ap()
    y_d = nc.dram_tensor("y", (T_DEC, BPC, N), FP, kind="ExternalOutput").ap()

    with tile.TileContext(nc) as tc:
        _body(tc, x_d, A_d, wl0, wl1, Wp_d, bp_d, y_d, t_enc, t_dec)
    nc.compile()
    return nc


def _body(tc, x_d, A_d, wl0, wl1, Wp_d, bp_d, y_d, t_enc, t_dec):
    nc = tc.nc
    ctx = ExitStack()
    P = ctx.enter_context
    const = P(tc.tile_pool(name="const", bufs=1))
    work = P(tc.tile_pool(name="work", bufs=1))
    pdiff = P(tc.tile_pool(name="pdiff", bufs=4, space="PSUM"))  # (128,512)
    pgate = P(tc.tile_pool(name="pgate", bufs=2, space="PSUM"))  # (128,512)
    ptr = P(tc.tile_pool(name="ptr", bufs=2, space="PSUM"))      # (128,512)

    def dps():
        return pdiff.tile([128, N], FP, tag="dps", name="dps")

    def gps():
        return pgate.tile([128, N], FP, tag="gps", name="gps")

    def tps():
        return ptr.tile([128, N], FP, tag="tps", name="tps")

    ident = const.tile([128, 128], FP)
    masks.make_identity(nc, ident[:])
    ones_col = const.tile([128, 1], FP)
    nc.gpsimd.memset(ones_col[:], 1.0)
    ones_row = const.tile([1, 128], FP)
    nc.gpsimd.memset(ones_row[:], 1.0)
    zeros = const.tile([128, N], FP)
    nc.gpsimd.memset(zeros[:], 0.0)

    def zinit(ap):
        p, f = ap.shape[0], ap.shape[-1]
        nc.vector.tensor_copy(rw(ap), zeros[0:p, 0:f])

    # ---------------- setup: random-walk matrices ----------------
    Arow = [const.tile([128, N], FP, name=f"Arow{i}") for i in range(NJC)]
    for i in range(NJC):
        nc.sync.dma_start(Arow[i][:], A_d[i * 128:(i + 1) * 128, :])

    Wfrow = [const.tile([128, N], FP, name=f"Wfrow{i}") for i in range(NJC)]
    for i in range(NJC):
        rs = const.tile([128, 1], FP, name=f"rs{i}")
        nc.vector.reduce_sum(rs[:], Arow[i][:], axis=mybir.AxisListType.X)
        nc.vector.tensor_scalar_add(rs[:], rs[:], 1e-6)
        nc.vector.reciprocal(rs[:], rs[:])
        nc.vector.tensor_scalar_mul(rw(Wfrow[i][:]), Arow[i][:], rs[:])

    # colsum -> inv -> broadcast (128, N)
    cs_ps = gps()
    for i in range(NJC):
        nc.tensor.matmul(cs_ps[0:1, :], ones_col[:], Arow[i][:],
                         start=(i == 0), stop=(i == NJC - 1))
    cs = const.tile([1, N], FP)
    nc.vector.tensor_scalar_add(cs[:], cs_ps[0:1, :], 1e-6)
    nc.vector.reciprocal(cs[:], cs[:])
    binv_ps = dps()
    nc.tensor.matmul(binv_ps[:], ones_row[:], cs[:])
    binv = const.tile([128, N], FP)
    nc.vector.tensor_copy(binv[:], binv_ps[:])

    WfT = [const.tile([128, N], FP, name=f"WfT{j}") for j in range(NJC)]
    WbT = [const.tile([128, N], FP, name=f"WbT{j}") for j in range(NJC)]
    for j in range(NJC):
        for i in range(NJC):
            tp = tps()
            nc.tensor.transpose(tp[:, 0:128],
                                Wfrow[i][:, j * 128:(j + 1) * 128], ident[:])
            nc.vector.tensor_copy(rw(WfT[j][:, i * 128:(i + 1) * 128]),
                                  tp[:, 0:128])
        nc.vector.tensor_mul(rw(WbT[j][:]), Arow[j][:], binv[:])

    Wf2T = [const.tile([128, N], FP, name=f"Wf2T{j}") for j in range(NJC)]
    Wb2T = [const.tile([128, N], FP, name=f"Wb2T{j}") for j in range(NJC)]
    for j in range(NJC):
        ps = dps()
        for m in range(NJC):
            nc.tensor.matmul(ps[:], rr(Wfrow[m][:, j * 128:(j + 1) * 128]),
                             rr(WfT[m][:]),
                             start=(m == 0), stop=(m == NJC - 1))
        nc.vector.tensor_copy(rw(Wf2T[j][:]), ps[:])
    for j in range(NJC):
        ps = dps()
        for m in range(NJC):
            tp = tps()
            nc.tensor.transpose(tp[:, 0:128],
                                WbT[j][:, m * 128:(m + 1) * 128], ident[:])
            tsb = work.tile([128, 128], FP, tag="setup_tsb", name="setup_tsb")
            nc.vector.tensor_copy(rw(tsb[:]), tp[:, 0:128])
            nc.tensor.matmul(ps[:], rr(tsb[:]), rr(WbT[m][:]),
                             start=(m == 0), stop=(m == NJC - 1))
        nc.vector.tensor_copy(rw(Wb2T[j][:]), ps[:])

    HOPS = [WfT, Wf2T, WbT, Wb2T]

    # ---------------- setup: gate weights ----------------
    # Layer 0 rows: 0=x, 1:65=h, 65=D1x, 66:130=D1h, 130=D2x, 131:195=D2h,
    #               195=D3x, 196:260=D3h, 260=D4x, 261:325=D4h
    # Layer 1 rows: hop-major blocks of 128 = [xl(=h0) 64 | h(=h1) 64]
    H = HID

    def l0_chunks(gates, width):
        """width = len(gates)*64; returns dict of packed L0 chunk tiles."""
        xh = const.tile([65, width], FP, name=f"W0xh_{gates}")
        ca = const.tile([128, width], FP, name=f"W0a_{gates}")
        cb = const.tile([128, width], FP, name=f"W0b_{gates}")
        cx = const.tile([97, width], FP, name=f"W0x_{gates}")
        for dst, zero in ((xh, True), (ca, False), (cb, False), (cx, True)):
            p = dst.shape[0]
            stg = work.tile([p, width], FP, tag="wstg0", name="wstg0")
            if zero:
                nc.vector.tensor_copy(stg[:], zeros[0:p, 0:width])
            for gi, g in enumerate(gates):
                W = wl0[g]
                c0, c1 = gi * H, (gi + 1) * H
                if dst is xh:
                    nc.sync.dma_start(stg[0:64, c0:c1], W[1:65, :])
                    nc.sync.dma_start(stg[64:65, c0:c1], W[0:1, :])
                elif dst is ca:
                    nc.sync.dma_start(stg[0:64, c0:c1], W[66:130, :])
                    nc.sync.dma_start(stg[64:128, c0:c1], W[131:195, :])
                elif dst is cb:
                    nc.sync.dma_start(stg[0:64, c0:c1], W[196:260, :])
                    nc.sync.dma_start(stg[64:128, c0:c1], W[261:325, :])
                else:
                    for k, r in enumerate([65, 130, 195, 260]):
                        nc.sync.dma_start(stg[32 * k:32 * k + 1, c0:c1],
                                          W[r:r + 1, :])
            nc.vector.tensor_copy(rw(dst[:]), stg[:])
        return dict(xh=xh, a=ca, b=cb, x=cx)

    def l1_chunks(gates, width):
        tiles = {}
        rows = {"id": [(0, 128, 0)],
                "h0a": [(0, 64, 128), (64, 128, 256)],
                "h0b": [(0, 64, 384), (64, 128, 512)],
                "h1a": [(0, 64, 192), (64, 128, 320)],
                "h1b": [(0, 64, 448), (64, 128, 576)]}
        for nm, rspec in rows.items():
            dst = const.tile([128, width], FP, name=f"W1{nm}_{gates}")
            stg = work.tile([128, width], FP, tag="wstg1", name="wstg1")
            for gi, g in enumerate(gates):
                W = wl1[g]
                c0, c1 = gi * H, (gi + 1) * H
                for r0, r1, wr in rspec:
                    nc.sync.dma_start(stg[r0:r1, c0:c1], W[wr:wr + (r1 - r0), :])
            nc.vector.tensor_copy(rw(dst[:]), stg[:])
            tiles[nm] = dst
        return tiles

    W0rz = l0_chunks("rz", 128)
    W0n = l0_chunks("n", 64)
    W1rz = l1_chunks("rz", 128)
    W1n = l1_chunks("n", 64)

    def bias2(name, top, bot):
        b = const.tile([128, 1], FP, name=name)
        nc.sync.dma_start(b[0:64, :], top.rearrange("(h o) -> h o", o=1))
        nc.sync.dma_start(b[64:128, :], bot.rearrange("(h o) -> h o", o=1))
        return b

    bias0rz = bias2("bias0rz", wl0["rb"], wl0["zb"])
    bias0n = bias2("bias0n", wl0["nb"], wl0["nb"])
    bias1rz = bias2("bias1rz", wl1["rb"], wl1["zb"])
    bias1n = bias2("bias1n", wl1["nb"], wl1["nb"])

    Wp128 = const.tile([128, 1], FP)
    WpPa = const.tile([128, 33], FP)
    WpPb = const.tile([128, 97], FP)
    wpstg = work.tile([128, 131], FP, tag="wpstg", name="wpstg")
    nc.vector.tensor_copy(wpstg[:], zeros[:, 0:131])
    # WpPa = stg[0:33): {0:[Wp;0], 32:[0;Wp]}
    # WpPb = stg[33:130): within-tile cols {64:[Wp;0], 96:[0;Wp]}
    nc.sync.dma_start(wpstg[0:64, 0:1], Wp_d[:])
    nc.sync.dma_start(wpstg[64:128, 32:33], Wp_d[:])
    nc.sync.dma_start(wpstg[0:64, 97:98], Wp_d[:])
    nc.sync.dma_start(wpstg[64:128, 129:130], Wp_d[:])
    nc.sync.dma_start(wpstg[0:64, 130:131], Wp_d[:])
    nc.sync.dma_start(wpstg[64:128, 130:131], Wp_d[:])
    nc.vector.tensor_copy(rw(WpPa[:]), wpstg[:, 0:33])
    nc.vector.tensor_copy(rw(WpPb[:]), wpstg[:, 33:130])
    nc.vector.tensor_copy(rw(Wp128[:]), wpstg[:, 130:131])
    bpv = const.tile([1, 1], FP)
    nc.sync.dma_start(bpv[:], bp_d.rearrange("(h o) -> h o", o=1))

    # ---------------- setup: encoder x stationaries ----------------
    # xgrp[g][jc]: (128,128), col 32*(2*(t%2)+b) = x[b, 2g+(t%2), jc nodes]
    NG = t_enc // 2
    xgrp = [[const.tile([128, 128], FP, name=f"xgrp{g}_{j}")
             for j in range(NJC)] for g in range(NG)]
    xgstg = const.tile([128, 128], FP, name="xgstg")
    zinit(xgstg[:])
    for g in range(NG):
        for j in range(NJC):
            for p in range(2):
                for b in range(BPC):
                    nc.sync.dma_start(
                        xgstg[:, 32 * (2 * p + b):32 * (2 * p + b) + 1],
                        x_d[b, 2 * g + p, j * 128:(j + 1) * 128]
                        .rearrange("(n o) -> n o", o=1))
            nc.vector.tensor_copy(rw(xgrp[g][j][:]), xgstg[:])

    # ---------------- state ----------------
    hcat = [const.tile([128, N], FP, name=f"hcat{b}") for b in range(BPC)]
    h0n = [const.tile([128, 128], FP, name=f"h0n{j}") for j in range(NJC)]
    sln = [const.tile([128, 128], FP, name=f"sln{j}") for j in range(NJC)]
    slr = [const.tile([128, 128], FP, name=f"slr{j}") for j in range(NJC)]
    rh0n = [const.tile([128, 128], FP, name=f"rh0n{j}") for j in range(NJC)]
    # per-batch pair tiles: [hop_i 0:64 ; hop_j 64:128]
    Ph0a = [const.tile([128, N], FP, name=f"Ph0a{b}") for b in range(BPC)]
    Ph0b = [const.tile([128, N], FP, name=f"Ph0b{b}") for b in range(BPC)]
    Ph1a = [const.tile([128, N], FP, name=f"Ph1a{b}") for b in range(BPC)]
    Ph1b = [const.tile([128, N], FP, name=f"Ph1b{b}") for b in range(BPC)]
    Pr0a = [const.tile([128, N], FP, name=f"Pr0a{b}") for b in range(BPC)]
    Pr0b = [const.tile([128, N], FP, name=f"Pr0b{b}") for b in range(BPC)]
    Pr1a = [const.tile([128, N], FP, name=f"Pr1a{b}") for b in range(BPC)]
    Pr1b = [const.tile([128, N], FP, name=f"Pr1b{b}") for b in range(BPC)]
    XH0 = [const.tile([65, N], FP, name=f"XH0{b}") for b in range(BPC)]
    xrow = [const.tile([1, N], FP, name=f"xrow{b}") for b in range(BPC)]
    XRH0 = [const.tile([65, N], FP, name=f"XRH0{b}") for b in range(BPC)]
    cxa = [[const.tile([97, N], FP, name=f"cxa{b}_{p}") for p in range(2)]
           for b in range(BPC)]
    RH = const.tile([128, N], FP, name="RH")
    XR = const.tile([128, N], FP, name="XR")
    H0S = const.tile([128, N], FP, name="H0S")
    H1S = const.tile([128, N], FP, name="H1S")
    xr_fm = [const.tile([128, N], FP, name=f"xrfm{b}") for b in range(BPC)]
    h1f0 = const.tile([64, N], FP, name="h1f0")
    yb = const.tile([97, (T_DEC * BPC // 4) * N], FP)

    zinit(h1f0[:])
    for b in range(BPC):
        zinit(hcat[b][:])
        zinit(Ph0a[b][:])
        zinit(Ph0b[b][:])
        for p in range(2):
            zinit(cxa[b][p][:])
    for j in range(NJC):
        zinit(h0n[j][:])
        zinit(sln[j][:])
    nc.gpsimd.memset(yb[:], 0.0)

    def row_ap(tile_, r):
        return tile_[32 * (r % 4):32 * (r % 4) + 1,
                     (r // 4) * N:(r // 4 + 1) * N]

    ACT = mybir.ActivationFunctionType

    def diffuse(lhsT_fn):
        """16 accumulating matmuls -> 4 hop PSUM tiles (128=2bx64, N)."""
        outs = [dps() for _ in range(4)]
        for jc in range(NJC):
            lhsT = rr(lhsT_fn(jc))
            for g in range(4):
                nc.tensor.matmul(outs[g][:], lhsT, rr(HOPS[g][jc][:]),
                                 start=(jc == 0), stop=(jc == NJC - 1))
        return outs

    def pair_copies(outs, pa, pb):
        """PSUM hop outputs -> per-batch pair tiles (8 copies)."""
        for b in range(BPC):
            sl = slice(b * 64, (b + 1) * 64)
            nc.vector.tensor_copy(rw(pa[b][0:64, :]), outs[0][sl, :])
            nc.vector.tensor_copy(rw(pb[b][0:64, :]), outs[2][sl, :])
            if b == 1:
                nc.vector.tensor_copy(rw(pa[b][64:128, :]), outs[1][sl, :])
                nc.vector.tensor_copy(rw(pb[b][64:128, :]), outs[3][sl, :])
            else:
                nc.scalar.copy(rw(pa[b][64:128, :]), outs[1][sl, :])
                nc.scalar.copy(rw(pb[b][64:128, :]), outs[3][sl, :])

    def gate_mms(ps_ap, chunks):
        n = len(chunks)
        for i, (w, x) in enumerate(chunks):
            nc.tensor.matmul(ps_ap, rr(w), rr(x),
                             start=(i == 0), stop=(i == n - 1))

    def transposeB(src64, dst_list, cb):
        """src64: (64,N) at base 64 -> 4 transposes; dst_list[jc] cols cb:cb+64."""
        tp = tps()
        for jc in range(NJC):
            nc.tensor.transpose(tp[:, jc * 64:(jc + 1) * 64],
                                src64[:, jc * 128:(jc + 1) * 128],
                                ident[64:128, 64:128])
        for jc in range(NJC):
            s = tp[:, jc * 64:(jc + 1) * 64]
            if jc % 2 == 0:
                nc.vector.tensor_copy(rw(dst_list[jc][:, cb:cb + 64]), s)
            else:
                nc.scalar.copy(rw(dst_list[jc][:, cb:cb + 64]), s)

    def transpose4(src, dsts):
        """src (128, N) stacked -> 4 PE transposes; dsts[jc] gets (128,128)."""
        tp = tps()
        for jc in range(NJC):
            nc.tensor.transpose(tp[:, jc * 128:(jc + 1) * 128],
                                src[:, jc * 128:(jc + 1) * 128],
                                ident[:])
        for jc in range(NJC):
            src = tp[:, jc * 128:(jc + 1) * 128]
            if jc % 2 == 0:
                nc.vector.tensor_copy(rw(dsts[jc]), src)
            else:
                nc.scalar.copy(rw(dsts[jc]), src)

    for t in range(t_enc + t_dec):
        dec = t >= t_enc
        td = t - t_enc

        # ---- (a) diffuse h1_{t-1} (slot) -> Ph1a/b ----
        if t > 0:
            outs = diffuse(lambda jc: sln[jc][:])
            pair_copies(outs, Ph1a, Ph1b)

        # ---- (b) x hops for this step ----
        if not dec:
            if t % 2 == 0:
                g = t // 2
                xo = [dps() for _ in range(4)]
                for jc in range(NJC):
                    lhsT = rr(xgrp[g][jc][:])
                    for h in range(4):
                        nc.tensor.matmul(xo[h][:], lhsT, rr(HOPS[h][jc][:]),
                                         start=(jc == 0), stop=(jc == NJC - 1))
                for p in range(2):
                    for b in range(BPC):
                        for h in range(4):
                            s = 32 * (2 * p + b)
                            nc.scalar.copy(
                                rw(cxa[b][p][32 * h:32 * h + 1, :]),
                                xo[h][s:s + 1, :])
            for b in range(BPC):
                nc.sync.dma_start(
                    xrow[b][:], x_d[b, t, :].rearrange("(o n) -> o n", o=1))
                nc.scalar.copy(rw(XH0[b][64:65, :]), xrow[b][:])
                nc.scalar.copy(rw(XRH0[b][64:65, :]), xrow[b][:])
            xsel = t % 2
        else:
            if td == 0:
                for b in range(BPC):
                    nc.sync.dma_start(
                        xrow[b][:],
                        x_d[b, t_enc - 1, :].rearrange("(o n) -> o n", o=1))
                    nc.scalar.copy(rw(XH0[b][64:65, :]), xrow[b][:])
                    nc.scalar.copy(rw(XRH0[b][64:65, :]), xrow[b][:])
                xsel = (t_enc - 1) % 2
            else:
                for b in range(BPC):
                    yrow = row_ap(yb, (td - 1) * BPC + b)
                    nc.scalar.copy(rw(XH0[b][64:65, :]), yrow)
                    nc.scalar.copy(rw(XRH0[b][64:65, :]), yrow)
                    psx = gps()
                    nc.tensor.matmul(psx[0:97, :], rr(WpPb[:]),
                                     rr(Ph1b[b][:]), start=True, stop=False)
                    nc.tensor.matmul(psx[0:33, :], rr(WpPa[:]),
                                     rr(Ph1a[b][:]), start=False, stop=True)
                    for h in range(4):
                        nc.scalar.activation(
                            rw(cxa[b][0][32 * h:32 * h + 1, :]),
                            psx[32 * h:32 * h + 1, :],
                            ACT.Identity, bias=bpv[:])
                xsel = 0

        # ---- (c) L0 r/z gates ----
        r0t, z0t = [], []
        for b in range(BPC):
            nc.vector.tensor_copy(rw(XH0[b][0:64, :]), hcat[b][0:64, :])
            ps = gps()
            ch = [(W0rz["xh"][:], XH0[b][:])]
            if t > 0:
                ch += [(W0rz["a"][:], Ph0a[b][:]), (W0rz["b"][:], Ph0b[b][:])]
            ch += [(W0rz["x"][:], cxa[b][xsel][:])]
            gate_mms(ps[:], ch)
            r = work.tile([64, N], FP, tag=f"r0t{b}", name=f"r0t{b}")
            z = work.tile([64, N], FP, tag=f"z0t{b}", name=f"z0t{b}")
            nc.scalar.activation(r[:], ps[0:64, :], ACT.Sigmoid,
                                 bias=bias0rz[0:64, :])
            nc.scalar.activation(z[:], ps[64:128, :], ACT.Sigmoid,
                                 bias=bias0rz[64:128, :])
            r0t.append(r)
            z0t.append(z)

        # ---- r*h0 (stacked; srcs base-aligned, cross-base dst ok) ----
        for b in range(BPC):
            nc.vector.tensor_mul(RH[b * 64:(b + 1) * 64, :],
                                 r0t[b][:], hcat[b][0:64, :])
        if t > 0:
            transpose4(RH[:], [rh0n[j][:] for j in range(NJC)])
            # ---- (e) diffuse r*h0 -> Pr0a/b ----
            outs = diffuse(lambda jc: rh0n[jc][:])
            pair_copies(outs, Pr0a, Pr0b)

        # ---- (f) L0 n gate + h0 update ----
        psn_b = []
        for b in range(BPC):
            psn = gps()
            psn_b.append(psn)
            if b == 0:
                nc.vector.tensor_copy(rw(XRH0[b][0:64, :]), RH[0:64, :])
            else:
                nc.scalar.copy(rw(XRH0[b][0:64, :]), RH[64:128, :])
            ch = [(W0n["xh"][:], XRH0[b][:])]
            if t > 0:
                ch += [(W0n["a"][:], Pr0a[b][:]), (W0n["b"][:], Pr0b[b][:])]
            ch += [(W0n["x"][:], cxa[b][xsel][:])]
            gate_mms(psn[0:64, :], ch)
        for b in range(BPC):
            n0 = work.tile([64, N], FP, tag=f"n0t{b}", name=f"n0t{b}")
            nc.scalar.activation(n0[:], psn_b[b][0:64, :], ACT.Tanh,
                                 bias=bias0n[0:64, :])
            sl = slice(b * 64, (b + 1) * 64)
            d = work.tile([64, N], FP, tag=f"d0{b}", name=f"d0{b}")
            nc.vector.tensor_sub(d[:], n0[:], hcat[b][0:64, :])
            nc.vector.tensor_mul(d[:], z0t[b][:], d[:])
            nc.vector.tensor_add(rw(hcat[b][0:64, :]), hcat[b][0:64, :], d[:])
            if b == 0:
                nc.vector.tensor_copy(H0S[sl, :], hcat[b][0:64, :])
            else:
                nc.scalar.copy(H0S[sl, :], hcat[b][0:64, :])
        transpose4(H0S[:], [h0n[j][:] for j in range(NJC)])

        # ---- (g) diffuse h0_t -> Ph0a/b (reused by L1 now, L0 at t+1) ----
        outs = diffuse(lambda jc: h0n[jc][:])
        pair_copies(outs, Ph0a, Ph0b)

        # ---- (h) L1 r/z gates ----
        r1t, z1t = [], []
        for b in range(BPC):
            ps = gps()
            ch = [(W1rz["id"][:], hcat[b][:]),
                  (W1rz["h0a"][:], Ph0a[b][:]), (W1rz["h0b"][:], Ph0b[b][:])]
            if t > 0:
                ch += [(W1rz["h1a"][:], Ph1a[b][:]),
                       (W1rz["h1b"][:], Ph1b[b][:])]
            gate_mms(ps[:], ch)
            r = work.tile([128, N], FP, tag=f"r1t{b}", name=f"r1t{b}")[64:128, :]
            z = work.tile([128, N], FP, tag=f"z1t{b}", name=f"z1t{b}")[64:128, :]
            nc.scalar.activation(r, ps[0:64, :], ACT.Sigmoid,
                                 bias=bias1rz[0:64, :])
            nc.scalar.activation(z, ps[64:128, :], ACT.Sigmoid,
                                 bias=bias1rz[64:128, :])
            r1t.append(r)
            z1t.append(z)
        for b in range(BPC):
            nc.vector.tensor_copy(rw(xr_fm[b][0:64, :]), hcat[b][0:64, :])
            nc.vector.tensor_mul(rw(xr_fm[b][64:128, :]),
                                 r1t[b], hcat[b][64:128, :])
        if t > 0:
            for b in range(BPC):
                transposeB(xr_fm[b][64:128, :], slr, b * 64)
            # ---- (j) diffuse r1*h1 -> Pr1a/b ----
            outs = diffuse(lambda jc: slr[jc][:])
            pair_copies(outs, Pr1a, Pr1b)

        # ---- (k) L1 n gate + h1 update ----
        psn1_b = []
        for b in range(BPC):
            psn1 = gps()
            psn1_b.append(psn1)
            ch = [(W1n["id"][:], xr_fm[b][:])]
            if t > 0:
                ch += [(W1n["h1a"][:], Pr1a[b][:]), (W1n["h1b"][:], Pr1b[b][:])]
            ch += [(W1n["h0a"][:], Ph0a[b][:]), (W1n["h0b"][:], Ph0b[b][:])]
            gate_mms(psn1[0:64, :], ch)
        for b in range(BPC):
            n1 = work.tile([128, N], FP, tag=f"n1t{b}", name=f"n1t{b}")[64:128, :]
            nc.scalar.activation(n1, psn1_b[b][0:64, :], ACT.Tanh,
                                 bias=bias1n[0:64, :])
            d = work.tile([128, N], FP, tag=f"d1{b}", name=f"d1{b}")[64:128, :]
            nc.vector.tensor_sub(d, n1, hcat[b][64:128, :])
            nc.vector.tensor_mul(d, z1t[b], d)
            nc.vector.tensor_add(rw(hcat[b][64:128, :]), hcat[b][64:128, :], d)
            transposeB(hcat[b][64:128, :], sln, b * 64)

        # ---- (l) decoder projection ----
        if dec:
            for b in range(BPC):
                psy = gps()
                nc.tensor.matmul(psy[0:1, :],
                                 rr(Wp128[64:128, :]), rr(hcat[b][64:128, :]))
                nc.scalar.activation(row_ap(yb, td * BPC + b),
                                     psy[0:1, :],
                                     ACT.Identity, bias=bpv[:])

    for td in range(t_dec):
        for b in range(BPC):
            nc.sync.dma_start(y_d[td, b:b + 1, :], row_ap(yb, td * BPC + b))
    ctx.close()


_cache = {}


def _get_module():
    if "nc" not in _cache:
        _cache["nc"] = build_module()
    return _cache["nc"]


def kernel(**inputs):
    x = np.asarray(inputs["x"], np.float32)
    nc = _get_module()
    names = ["A", "Wr0", "br0", "Wz0", "bz0", "Wn0", "bn0",
             "Wr1", "br1", "Wz1", "bz1", "Wn1", "bn1", "Wp", "bp"]
    shared = {n: np.asarray(inputs[n], np.float32) for n in names}
    in_maps = []
    for c in range(NC):
        m = dict(shared)
        m["x"] = x[c * BPC:(c + 1) * BPC]
        in_maps.append(m)
    res = run_bass_kernel_spmd(nc, in_maps, core_ids=list(range(NC)))
    outs = []
    for c in range(NC):
        y = res.results[c]["y"]            # (T_DEC, BPC, N)
        outs.append(np.moveaxis(y, 0, 1))  # (BPC, T_DEC, N)
    return np.concatenate(outs, axis=0).astype(np.float32)


# revision 26
# speedup vs baseline: 1.0586x; 1.0586x over previous
"""DCRNN (nn_DCRNNModel_35837207118645) Trainium2 Bass kernel, v2.

Data-parallel over batch: B=16 -> 2 per core x 8 cores; 24 recurrent steps
fully unrolled, everything resident in SBUF.

Key structure (vs v1):
  - Split-by-half diffusion: W^k is applied to h0, h1, r*h0, r1*h1 halves
    separately (stationary = node-major halves of BOTH batches packed to
    128 cols).  D(h0_t) computed for layer-1 of step t is reused by
    layer-0 of step t+1, removing one of the four diffusion rounds.
  - r+z gate weights packed side by side -> one matmul stream serves both
    gates; n-gate matmuls for the two batch items write disjoint
    partition halves of one PSUM tile (one tanh for both).
  - Encoder x-hops are computed in 2-step groups (4 stationary columns at
    partitions {0,32,64,96}); decoder x-hops are derived algebraically:
    W^k y = (W^k h1) @ Wp + bp, reusing the already-diffused h1 halves.
"""

from contextlib import ExitStack

import numpy as np

import concourse.bass as bass
import concourse.bacc as bacc
import concourse.tile as tile
import concourse.mybir as mybir
from concourse import masks
from concourse.bass_utils import run_bass_kernel_spmd

FP = mybir.dt.float32
R32 = mybir.dt.float32r


def rr(ap):
    return ap.bitcast(R32)


def rw(ap):
    # round-on-write marker for producers feeding f32r matmuls
    return ap.bitcast(R32)


N = 512
HID = 64
NC = 8          # cores
BPC = 2         # batch per core
T_ENC = 12
T_DEC = 12
NJC = N // 128  # 4 node chunks


def build_module(t_enc=T_ENC, t_dec=T_DEC):
    nc = bacc.Bacc("TRN2", target_bir_lowering=False, debug=False)

    x_d = nc.dram_tensor("x", (BPC, T_ENC, N), FP, kind="ExternalInput").ap()
    A_d = nc.dram_tensor("A", (N, N), FP, kind="ExternalInput").ap()
    wl0, wl1 = {}, {}
    for g in "rzn":
        wl0[g] = nc.dram_tensor(f"W{g}0", (325, HID), FP, kind="ExternalInput").ap()
        wl0[g + "b"] = nc.dram_tensor(f"b{g}0", (HID,), FP, kind="ExternalInput").ap()
        wl1[g] = nc.dram_tensor(f"W{g}1", (640, HID), FP, kind="ExternalInput").ap()
        wl1[g + "b"] = nc.dram_tensor(f"b{g}1", (HID,), FP, kind="ExternalInput").ap()
    Wp_d = nc.dram_tensor("Wp", (HID, 1), FP, kind="ExternalInput").ap()
    bp_d = nc.dram_tensor("bp", (1,), FP, kind="ExternalInput").ap()
    y_d = nc.dram_tensor("y", (T_DEC, BPC, N), FP, kind="ExternalOutput").ap()

    with tile.TileContext(nc) as tc:
        _body(tc, x_d, A_d, wl0, wl1, Wp_d, bp_d, y_d, t_enc, t_dec)
    nc.compile()
    return nc


def _body(tc, x_d, A_d, wl0, wl1, Wp_d, bp_d, y_d, t_enc, t_dec):
    nc = tc.nc
    ctx = ExitStack()
    P = ctx.enter_context
    const = P(tc.tile_pool(name="const", bufs=1))
    work = P(tc.tile_pool(name="work", bufs=1))
    pdiff = P(tc.tile_pool(name="pdiff", bufs=4, space="PSUM"))  # (128,512)
    pgate = P(tc.tile_pool(name="pgate", bufs=2, space="PSUM"))  # (128,512)
    ptr = P(tc.tile_pool(name="ptr", bufs=2, space="PSUM"))      # (128,512)

    def dps():
        return pdiff.tile([128, N], FP, tag="dps", name="dps")

    def gps():
        return pgate.tile([128, N], FP, tag="gps", name="gps")

    def tps():
        return ptr.tile([128, N], FP, tag="tps", name="tps")

    ident = const.tile([128, 128], FP)
    masks.make_identity(nc, ident[:])
    ones_col = const.tile([128, 1], FP)
    nc.gpsimd.memset(ones_col[:], 1.0)
    ones_row = const.tile([1, 128], FP)
    nc.gpsimd.memset(ones_row[:], 1.0)
    zeros = const.tile([128, N], FP)
    nc.gpsimd.memset(zeros[:], 0.0)

    def zinit(ap):
        p, f = ap.shape[0], ap.shape[-1]
        nc.vector.tensor_copy(rw(ap), zeros[0:p, 0:f])

    # ---------------- setup: random-walk matrices ----------------
    Arow = [const.tile([128, N], FP, name=f"Arow{i}") for i in range(NJC)]
    for i in range(NJC):
        nc.sync.dma_start(Arow[i][:], A_d[i * 128:(i + 1) * 128, :])

    Wfrow = [const.tile([128, N], FP, name=f"Wfrow{i}") for i in range(NJC)]
    for i in range(NJC):
        rs = const.tile([128, 1], FP, name=f"rs{i}")
        nc.vector.reduce_sum(rs[:], Arow[i][:], axis=mybir.AxisListType.X)
        nc.vector.tensor_scalar_add(rs[:], rs[:], 1e-6)
        nc.vector.reciprocal(rs[:], rs[:])
        nc.vector.tensor_scalar_mul(rw(Wfrow[i][:]), Arow[i][:], rs[:])

    # colsum -> inv -> broadcast (128, N)
    cs_ps = gps()
    for i in range(NJC):
        nc.tensor.matmul(cs_ps[0:1, :], ones_col[:], Arow[i][:],
                         start=(i == 0), stop=(i == NJC - 1))
    cs = const.tile([1, N], FP)
    nc.vector.tensor_scalar_add(cs[:], cs_ps[0:1, :], 1e-6)
    nc.vector.reciprocal(cs[:], cs[:])
    binv_ps = dps()
    nc.tensor.matmul(binv_ps[:], ones_row[:], cs[:])
    binv = const.tile([128, N], FP)
    nc.vector.tensor_copy(binv[:], binv_ps[:])

    WfT = [const.tile([128, N], FP, name=f"WfT{j}") for j in range(NJC)]
    WbT = [const.tile([128, N], FP, name=f"WbT{j}") for j in range(NJC)]
    for j in range(NJC):
        for i in range(NJC):
            tp = tps()
            nc.tensor.transpose(tp[:, 0:128],
                                Wfrow[i][:, j * 128:(j + 1) * 128], ident[:])
            nc.vector.tensor_copy(rw(WfT[j][:, i * 128:(i + 1) * 128]),
                                  tp[:, 0:128])
        nc.vector.tensor_mul(rw(WbT[j][:]), Arow[j][:], binv[:])

    Wf2T = [const.tile([128, N], FP, name=f"Wf2T{j}") for j in range(NJC)]
    Wb2T = [const.tile([128, N], FP, name=f"Wb2T{j}") for j in range(NJC)]
    for j in range(NJC):
        ps = dps()
        for m in range(NJC):
            nc.tensor.matmul(ps[:], rr(Wfrow[m][:, j * 128:(j + 1) * 128]),
                             rr(WfT[m][:]),
                             start=(m == 0), stop=(m == NJC - 1))
        nc.vector.tensor_copy(rw(Wf2T[j][:]), ps[:])
    for j in range(NJC):
        ps = dps()
        for m in range(NJC):
            tp = tps()
            nc.tensor.transpose(tp[:, 0:128],
                                WbT[j][:, m * 128:(m + 1) * 128], ident[:])
            tsb = work.tile([128, 128], FP, tag="setup_tsb", name="setup_tsb")
            nc.vector.tensor_copy(rw(tsb[:]), tp[:, 0:128])
            nc.tensor.matmul(ps[:], rr(tsb[:]), rr(WbT[m][:]),
                             start=(m == 0), stop=(m == NJC - 1))
        nc.vector.tensor_copy(rw(Wb2T[j][:]), ps[:])

    HOPS = [WfT, Wf2T, WbT, Wb2T]

    # ---------------- setup: gate weights ----------------
    # Layer 0 rows: 0=x, 1:65=h, 65=D1x, 66:130=D1h, 130=D2x, 131:195=D2h,
    #               195=D3x, 196:260=D3h, 260=D4x, 261:325=D4h
    # Layer 1 rows: hop-major blocks of 128 = [xl(=h0) 64 | h(=h1) 64]
    H = HID

    def l0_chunks(gates, width):
        """width = len(gates)*64; returns dict of packed L0 chunk tiles."""
        xh = const.tile([65, width], FP, name=f"W0xh_{gates}")
        ca = const.tile([128, width], FP, name=f"W0a_{gates}")
        cb = const.tile([128, width], FP, name=f"W0b_{gates}")
        cx = const.tile([97, width], FP, name=f"W0x_{gates}")
        for dst, zero in ((xh, True), (ca, False), (cb, False), (cx, True)):
            p = dst.shape[0]
            stg = work.tile([p, width], FP, tag="wstg0", name="wstg0")
            if zero:
                nc.vector.tensor_copy(stg[:], zeros[0:p, 0:width])
            for gi, g in enumerate(gates):
                W = wl0[g]
                c0, c1 = gi * H, (gi + 1) * H
                if dst is xh:
                    nc.sync.dma_start(stg[0:64, c0:c1], W[1:65, :])
                    nc.sync.dma_start(stg[64:65, c0:c1], W[0:1, :])
                elif dst is ca:
                    nc.sync.dma_start(stg[0:64, c0:c1], W[66:130, :])
                    nc.sync.dma_start(stg[64:128, c0:c1], W[131:195, :])
                elif dst is cb:
                    nc.sync.dma_start(stg[0:64, c0:c1], W[196:260, :])
                    nc.sync.dma_start(stg[64:128, c0:c1], W[261:325, :])
                else:
                    for k, r in enumerate([65, 130, 195, 260]):
                        nc.sync.dma_start(stg[32 * k:32 * k + 1, c0:c1],
                                          W[r:r + 1, :])
            nc.vector.tensor_copy(rw(dst[:]), stg[:])
        return dict(xh=xh, a=ca, b=cb, x=cx)

    def l1_chunks(gates, width):
        tiles = {}
        rows = {"id": [(0, 128, 0)],
                "h0a": [(0, 64, 128), (64, 128, 256)],
                "h0b": [(0, 64, 384), (64, 128, 512)],
                "h1a": [(0, 64, 192), (64, 128, 320)],
                "h1b": [(0, 64, 448), (64, 128, 576)]}
        for nm, rspec in rows.items():
            dst = const.tile([128, width], FP, name=f"W1{nm}_{gates}")
            stg = work.tile([128, width], FP, tag="wstg1", name="wstg1")
            for gi, g in enumerate(gates):
                W = wl1[g]
                c0, c1 = gi * H, (gi + 1) * H
                for r0, r1, wr in rspec:
                    nc.sync.dma_start(stg[r0:r1, c0:c1], W[wr:wr + (r1 - r0), :])
            nc.vector.tensor_copy(rw(dst[:]), stg[:])
            tiles[nm] = dst
        return tiles

    W0rz = l0_chunks("rz", 128)
    W0n = l0_chunks("n", 64)
    W1rz = l1_chunks("rz", 128)
    W1n = l1_chunks("n", 64)

    def bias2(name, top, bot):
        b = const.tile([128, 1], FP, name=name)
        nc.sync.dma_start(b[0:64, :], top.rearrange("(h o) -> h o", o=1))
        nc.sync.dma_start(b[64:128, :], bot.rearrange("(h o) -> h o", o=1))
        return b

    bias0rz = bias2("bias0rz", wl0["rb"], wl0["zb"])
    bias0n = bias2("bias0n", wl0["nb"], wl0["nb"])
    bias1rz = bias2("bias1rz", wl1["rb"], wl1["zb"])
    bias1n = bias2("bias1n", wl1["nb"], wl1["nb"])

    Wp128 = const.tile([128, 1], FP)
    WpPa = const.tile([128, 33], FP)
    WpPb = const.tile([128, 97], FP)
    wpstg = work.tile([128, 131], FP, tag="wpstg", name="wpstg")
    nc.vector.tensor_copy(wpstg[:], zeros[:, 0:131])
    # WpPa = stg[0:33): {0:[Wp;0], 32:[0;Wp]}
    # WpPb = stg[33:130): within-tile cols {64:[Wp;0], 96:[0;Wp]}
    nc.sync.dma_start(wpstg[0:64, 0:1], Wp_d[:])
    nc.sync.dma_start(wpstg[64:128, 32:33], Wp_d[:])
    nc.sync.dma_start(wpstg[0:64, 97:98], Wp_d[:])
    nc.sync.dma_start(wpstg[64:128, 129:130], Wp_d[:])
    nc.sync.dma_start(wpstg[0:64, 130:131], Wp_d[:])
    nc.sync.dma_start(wpstg[64:128, 130:131], Wp_d[:])
    nc.vector.tensor_copy(rw(WpPa[:]), wpstg[:, 0:33])
    nc.vector.tensor_copy(rw(WpPb[:]), wpstg[:, 33:130])
    nc.vector.tensor_copy(rw(Wp128[:]), wpstg[:, 130:131])
    bpv = const.tile([1, 1], FP)
    nc.sync.dma_start(bpv[:], bp_d.rearrange("(h o) -> h o", o=1))

    # ---------------- setup: encoder x stationaries ----------------
    # xgrp[g][jc]: (128,128), col 32*(2*(t%2)+b) = x[b, 2g+(t%2), jc nodes]
    NG = t_enc // 2
    xgrp = [[const.tile([128, 128], FP, name=f"xgrp{g}_{j}")
             for j in range(NJC)] for g in range(NG)]
    xgstg = const.tile([128, 128], FP, name="xgstg")
    zinit(xgstg[:])
    for g in range(NG):
        for j in range(NJC):
            for p in range(2):
                for b in range(BPC):
                    nc.sync.dma_start(
                        xgstg[:, 32 * (2 * p + b):32 * (2 * p + b) + 1],
                        x_d[b, 2 * g + p, j * 128:(j + 1) * 128]
                        .rearrange("(n o) -> n o", o=1))
            nc.vector.tensor_copy(rw(xgrp[g][j][:]), xgstg[:])

    # ---------------- state ----------------
    hcat = [const.tile([128, N], FP, name=f"hcat{b}") for b in range(BPC)]
    h0n = [const.tile([128, 128], FP, name=f"h0n{j}") for j in range(NJC)]
    sln = [const.tile([128, 128], FP, name=f"sln{j}") for j in range(NJC)]
    slr = [const.tile([128, 128], FP, name=f"slr{j}") for j in range(NJC)]
    rh0n = [const.tile([128, 128], FP, name=f"rh0n{j}") for j in range(NJC)]
    # per-batch pair tiles: [hop_i 0:64 ; hop_j 64:128]
    Ph0a = [const.tile([128, N], FP, name=f"Ph0a{b}") for b in range(BPC)]
    Ph0b = [const.tile([128, N], FP, name=f"Ph0b{b}") for b in range(BPC)]
    Ph1a = [const.tile([128, N], FP, name=f"Ph1a{b}") for b in range(BPC)]
    Ph1b = [const.tile([128, N], FP, name=f"Ph1b{b}") for b in range(BPC)]
    Pr0a = [const.tile([128, N], FP, name=f"Pr0a{b}") for b in range(BPC)]
    Pr0b = [const.tile([128, N], FP, name=f"Pr0b{b}") for b in range(BPC)]
    Pr1a = [const.tile([128, N], FP, name=f"Pr1a{b}") for b in range(BPC)]
    Pr1b = [const.tile([128, N], FP, name=f"Pr1b{b}") for b in range(BPC)]
    XH0 = [const.tile([65, N], FP, name=f"XH0{b}") for b in range(BPC)]
    xrow = [const.tile([1, N], FP, name=f"xrow{b}") for b in range(BPC)]
    XRH0 = [const.tile([65, N], FP, name=f"XRH0{b}") for b in range(BPC)]
    cxa = [[const.tile([97, N], FP, name=f"cxa{b}_{p}") for p in range(2)]
           for b in range(BPC)]
    RH = const.tile([128, N], FP, name="RH")
    XR = const.tile([128, N], FP, name="XR")
    H0S = const.tile([128, N], FP, name="H0S")
    H1S = const.tile([128, N], FP, name="H1S")
    xr_fm = [const.tile([128, N], FP, name=f"xrfm{b}") for b in range(BPC)]
    h1f0 = const.tile([64, N], FP, name="h1f0")
    yb = const.tile([97, (T_DEC * BPC // 4) * N], FP)

    zinit(h1f0[:])
    for b in range(BPC):
        zinit(hcat[b][:])
        zinit(Ph0a[b][:])
        zinit(Ph0b[b][:])
        for p in range(2):
            zinit(cxa[b][p][:])
    for j in range(NJC):
        zinit(h0n[j][:])
        zinit(sln[j][:])
    nc.gpsimd.memset(yb[:], 0.0)

    def row_ap(tile_, r):
        return tile_[32 * (r % 4):32 * (r % 4) + 1,
                     (r // 4) * N:(r // 4 + 1) * N]

    ACT = mybir.ActivationFunctionType

    def diffuse(lhsT_fn):
        """16 accumulating matmuls -> 4 hop PSUM tiles (128=2bx64, N)."""
        outs = [dps() for _ in range(4)]
        for jc in range(NJC):
            lhsT = rr(lhsT_fn(jc))
            for g in range(4):
                nc.tensor.matmul(outs[g][:], lhsT, rr(HOPS[g][jc][:]),
                                 start=(jc == 0), stop=(jc == NJC - 1))
        return outs

    def pair_copies(outs, pa, pb):
        """PSUM hop outputs -> per-batch pair tiles (8 copies)."""
        for b in range(BPC):
            sl = slice(b * 64, (b + 1) * 64)
            nc.vector.tensor_copy(rw(pa[b][0:64, :]), outs[0][sl, :])
            nc.scalar.copy(rw(pa[b][64:128, :]), outs[1][sl, :])
            nc.vector.tensor_copy(rw(pb[b][0:64, :]), outs[2][sl, :])
            nc.scalar.copy(rw(pb[b][64:128, :]), outs[3][sl, :])

    def gate_mms(ps_ap, chunks):
        n = len(chunks)
        for i, (w, x) in enumerate(chunks):
            nc.tensor.matmul(ps_ap, rr(w), rr(x),
                             start=(i == 0), stop=(i == n - 1))

    def transposeB(src64, dst_list, cb):
        """src64: (64,N) at base 64 -> 4 transposes; dst_list[jc] cols cb:cb+64."""
        tp = tps()
        for jc in range(NJC):
            nc.tensor.transpose(tp[:, jc * 64:(jc + 1) * 64],
                                src64[:, jc * 128:(jc + 1) * 128],
                                ident[64:128, 64:128])
        for jc in range(NJC):
            s = tp[:, jc * 64:(jc + 1) * 64]
            if jc % 2 == 0:
                nc.vector.tensor_copy(rw(dst_list[jc][:, cb:cb + 64]), s)
            else:
                nc.scalar.copy(rw(dst_list[jc][:, cb:cb + 64]), s)

    def transpose4(src, dsts):
        """src (128, N) stacked -> 4 PE transposes; dsts[jc] gets (128,128)."""
        tp = tps()
        for jc in range(NJC):
            nc.tensor.transpose(tp[:, jc * 128:(jc + 1) * 128],
                                src[:, jc * 128:(jc + 1) * 128],
                                ident[:])
        for jc in range(NJC):
            src = tp[:, jc * 128:(jc + 1) * 128]
            if jc % 2 == 0:
                nc.vector.tensor_copy(rw(dsts[jc]), src)
            else:
                nc.scalar.copy(rw(dsts[jc]), src)

    for t in range(t_enc + t_dec):
        dec = t >= t_enc
        td = t - t_enc

        # ---- (a) diffuse h1_{t-1} (slot) -> Ph1a/b ----
        if t > 0:
            outs = diffuse(lambda jc: sln[jc][:])
            pair_copies(outs, Ph1a, Ph1b)

        # ---- (b) x hops for this step ----
        if not dec:
            if t % 2 == 0:
                g = t // 2
                xo = [dps() for _ in range(4)]
                for jc in range(NJC):
                    lhsT = rr(xgrp[g][jc][:])
                    for h in range(4):
                        nc.tensor.matmul(xo[h][:], lhsT, rr(HOPS[h][jc][:]),
                                         start=(jc == 0), stop=(jc == NJC - 1))
                for p in range(2):
                    for b in range(BPC):
                        for h in range(4):
                            s = 32 * (2 * p + b)
                            nc.scalar.copy(
                                rw(cxa[b][p][32 * h:32 * h + 1, :]),
                                xo[h][s:s + 1, :])
            for b in range(BPC):
                nc.sync.dma_start(
                    xrow[b][:], x_d[b, t, :].rearrange("(o n) -> o n", o=1))
                nc.scalar.copy(rw(XH0[b][64:65, :]), xrow[b][:])
                nc.scalar.copy(rw(XRH0[b][64:65, :]), xrow[b][:])
            xsel = t % 2
        else:
            if td == 0:
                for b in range(BPC):
                    nc.sync.dma_start(
                        xrow[b][:],
                        x_d[b, t_enc - 1, :].rearrange("(o n) -> o n", o=1))
                    nc.scalar.copy(rw(XH0[b][64:65, :]), xrow[b][:])
                    nc.scalar.copy(rw(XRH0[b][64:65, :]), xrow[b][:])
                xsel = (t_enc - 1) % 2
            else:
                for b in range(BPC):
                    yrow = row_ap(yb, (td - 1) * BPC + b)
                    nc.scalar.copy(rw(XH0[b][64:65, :]), yrow)
                    nc.scalar.copy(rw(XRH0[b][64:65, :]), yrow)
                    psx = gps()
                    nc.tensor.matmul(psx[0:97, :], rr(WpPb[:]),
                                     rr(Ph1b[b][:]), start=True, stop=False)
                    nc.tensor.matmul(psx[0:33, :], rr(WpPa[:]),
                                     rr(Ph1a[b][:]), start=False, stop=True)
                    for h in range(4):
                        nc.scalar.activation(
                            rw(cxa[b][0][32 * h:32 * h + 1, :]),
                            psx[32 * h:32 * h + 1, :],
                            ACT.Identity, bias=bpv[:])
                xsel = 0

        # ---- (c) L0 r/z gates ----
        r0t, z0t = [], []
        for b in range(BPC):
            nc.scalar.copy(rw(XH0[b][0:64, :]), hcat[b][0:64, :])
            ps = gps()
            ch = [(W0rz["xh"][:], XH0[b][:])]
            if t > 0:
                ch += [(W0rz["a"][:], Ph0a[b][:]), (W0rz["b"][:], Ph0b[b][:])]
            ch += [(W0rz["x"][:], cxa[b][xsel][:])]
            gate_mms(ps[:], ch)
            r = work.tile([64, N], FP, tag=f"r0t{b}", name=f"r0t{b}")
            z = work.tile([64, N], FP, tag=f"z0t{b}", name=f"z0t{b}")
            nc.scalar.activation(r[:], ps[0:64, :], ACT.Sigmoid,
                                 bias=bias0rz[0:64, :])
            nc.scalar.activation(z[:], ps[64:128, :], ACT.Sigmoid,
                                 bias=bias0rz[64:128, :])
            r0t.append(r)
            z0t.append(z)

        # ---- r*h0 (stacked; srcs base-aligned, cross-base dst ok) ----
        for b in range(BPC):
            nc.vector.tensor_mul(RH[b * 64:(b + 1) * 64, :],
                                 r0t[b][:], hcat[b][0:64, :])
        if t > 0:
            transpose4(RH[:], [rh0n[j][:] for j in range(NJC)])
            # ---- (e) diffuse r*h0 -> Pr0a/b ----
            outs = diffuse(lambda jc: rh0n[jc][:])
            pair_copies(outs, Pr0a, Pr0b)

        # ---- (f) L0 n gate + h0 update ----
        psn_b = []
        for b in range(BPC):
            psn = gps()
            psn_b.append(psn)
            if b == 0:
                nc.vector.tensor_copy(rw(XRH0[b][0:64, :]), RH[0:64, :])
            else:
                nc.scalar.copy(rw(XRH0[b][0:64, :]), RH[64:128, :])
            ch = [(W0n["xh"][:], XRH0[b][:])]
            if t > 0:
                ch += [(W0n["a"][:], Pr0a[b][:]), (W0n["b"][:], Pr0b[b][:])]
            ch += [(W0n["x"][:], cxa[b][xsel][:])]
            gate_mms(psn[0:64, :], ch)
        for b in range(BPC):
            n0 = work.tile([64, N], FP, tag=f"n0t{b}", name=f"n0t{b}")
            nc.scalar.activation(n0[:], psn_b[b][0:64, :], ACT.Tanh,
                                 bias=bias0n[0:64, :])
            sl = slice(b * 64, (b + 1) * 64)
            d = work.tile([64, N], FP, tag=f"d0{b}", name=f"d0{b}")
            nc.vector.tensor_sub(d[:], n0[:], hcat[b][0:64, :])
            nc.vector.tensor_mul(d[:], z0t[b][:], d[:])
            nc.vector.tensor_add(rw(hcat[b][0:64, :]), hcat[b][0:64, :], d[:])
            if b == 0:
                nc.vector.tensor_copy(H0S[sl, :], hcat[b][0:64, :])
            else:
                nc.scalar.copy(H0S[sl, :], hcat[b][0:64, :])
        transpose4(H0S[:], [h0n[j][:] for j in range(NJC)])

        # ---- (g) diffuse h0_t -> Ph0a/b (reused by L1 now, L0 at t+1) ----
        outs = diffuse(lambda jc: h0n[jc][:])
        pair_copies(outs, Ph0a, Ph0b)

        # ---- (h) L1 r/z gates ----
        r1t, z1t = [], []
        for b in range(BPC):
            ps = gps()
            ch = [(W1rz["id"][:], hcat[b][:]),
                  (W1rz["h0a"][:], Ph0a[b][:]), (W1rz["h0b"][:], Ph0b[b][:])]
            if t > 0:
                ch += [(W1rz["h1a"][:], Ph1a[b][:]),
                       (W1rz["h1b"][:], Ph1b[b][:])]
            gate_mms(ps[:], ch)
            r = work.tile([128, N], FP, tag=f"r1t{b}", name=f"r1t{b}")[64:128, :]
            z = work.tile([128, N], FP, tag=f"z1t{b}", name=f"z1t{b}")[64:128, :]
            nc.scalar.activation(r, ps[0:64, :], ACT.Sigmoid,
                                 bias=bias1rz[0:64, :])
            nc.scalar.activation(z, ps[64:128, :], ACT.Sigmoid,
                                 bias=bias1rz[64:128, :])
            r1t.append(r)
            z1t.append(z)
        for b in range(BPC):
            nc.scalar.copy(rw(xr_fm[b][0:64, :]), hcat[b][0:64, :])
            nc.vector.tensor_mul(rw(xr_fm[b][64:128, :]),
                                 r1t[b], hcat[b][64:128, :])
        if t > 0:
            for b in range(BPC):
                transposeB(xr_fm[b][64:128, :], slr, b * 64)
            # ---- (j) diffuse r1*h1 -> Pr1a/b ----
            outs = diffuse(lambda jc: slr[jc][:])
            pair_copies(outs, Pr1a, Pr1b)

        # ---- (k) L1 n gate + h1 update ----
        psn1_b = []
        for b in range(BPC):
            psn1 = gps()
            psn1_b.append(psn1)
            ch = [(W1n["id"][:], xr_fm[b][:])]
            if t > 0:
                ch += [(W1n["h1a"][:], Pr1a[b][:]), (W1n["h1b"][:], Pr1b[b][:])]
            ch += [(W1n["h0a"][:], Ph0a[b][:]), (W1n["h0b"][:], Ph0b[b][:])]
            gate_mms(psn1[0:64, :], ch)
        for b in range(BPC):
            n1 = work.tile([128, N], FP, tag=f"n1t{b}", name=f"n1t{b}")[64:128, :]
            nc.scalar.activation(n1, psn1_b[b][0:64, :], ACT.Tanh,
                                 bias=bias1n[0:64, :])
            d = work.tile([128, N], FP, tag=f"d1{b}", name=f"d1{b}")[64:128, :]
            nc.vector.tensor_sub(d, n1, hcat[b][64:128, :])
            nc.vector.tensor_mul(d, z1t[b], d)
            nc.vector.tensor_add(rw(hcat[b][64:128, :]), hcat[b][64:128, :], d)
            transposeB(hcat[b][64:128, :], sln, b * 64)

        # ---- (l) decoder projection ----
        if dec:
            for b in range(BPC):
                psy = gps()
                nc.tensor.matmul(psy[0:1, :],
                                 rr(Wp128[64:128, :]), rr(hcat[b][64:128, :]))
                nc.scalar.activation(row_ap(yb, td * BPC + b),
                                     psy[0:1, :],
                                     ACT.Identity, bias=bpv[:])

    for td in range(t_dec):
        for b in range(BPC):
            nc.sync.dma_start(y_d[td, b:b + 1, :], row_ap(yb, td * BPC + b))
    ctx.close()


_cache = {}


def _get_module():
    if "nc" not in _cache:
        _cache["nc"] = build_module()
    return _cache["nc"]


def kernel(**inputs):
    x = np.asarray(inputs["x"], np.float32)
    nc = _get_module()
    names = ["A", "Wr0", "br0", "Wz0", "bz0", "Wn0", "bn0",
             "Wr1", "br1", "Wz1", "bz1", "Wn1", "bn1", "Wp", "bp"]
    shared = {n: np.asarray(inputs[n], np.float32) for n in names}
    in_maps = []
    for c in range(NC):
        m = dict(shared)
        m["x"] = x[c * BPC:(c + 1) * BPC]
        in_maps.append(m)
    res = run_bass_kernel_spmd(nc, in_maps, core_ids=list(range(NC)))
    outs = []
    for c in range(NC):
        y = res.results[c]["y"]            # (T_DEC, BPC, N)
        outs.append(np.moveaxis(y, 0, 1))  # (BPC, T_DEC, N)
    return np.concatenate(outs, axis=0).astype(np.float32)
